# revision 13
# baseline (speedup 1.0000x reference)
"""Trainium2 Bass kernel for nn_Clustering_80900003987951 (vq_codebook).

Math (reference):
  x: [B=128, S=128, F=64, 1], centroids: [1, K=64, S=128, F=64]
  d2[b,k,s] = sum_f (x[b,s,f] - c[k,s,f])^2
  dist[b,k] = sum_s sqrt(d2[b,k,s])
  q = (1 + dist^2/2)^-3, normalized over k                  -> [B, K]

Strategy: shard the SEQUENCE dim across the 8 cores (S_loc=16), keep the
full batch on every core. Per-core input drops to ~200KB (vs 1.36MB for
batch sharding, where every core must load all centroids), matmuls use
all 128 output partitions, and the device returns two per-core partial
sums of sqrt(d2) over its s-shard (split so the final DMA depends only
on the short last PSUM bank). The host sums the 16 partials and applies
the tiny q tail (25K flops, ~0.002% of the work) exactly in float64.

Device pipeline per core:
  xt [66, *]: rows 0-63 = x^T (F on partitions), 64 = 1, 65 = |x|^2
  ct [66, *]: rows 0-63 = -2*c^T, 64 = |c|^2, 65 = 1
  per s: d2 tile = xt_s^T @ ct_s -> PSUM [128,64], ONE fp8 matmul per s
  (uniform weight dtype — alternating fp8/fp16 weights measured 325ns/s
  because it breaks ldweights/matmul pipelining, vs 60ns uniform; the
  part of the fp8 quantization error that is constant across k cancels
  in the normalized output; measured 6e-3 vs the 2e-2 budget).
  16 s split over 4 PSUM banks, skewed (5,5,4,2) so after the final
  matmul only sqrt[128,128] + one DVE pair-add + its own DMA remain; ACT
  sqrt per bank -> fp16 sbuf; contiguous fp16 add-trees + accumulator on
  DVE (strided tensor_reduce measured 1.8ns/elem vs ~0.5 here). A dummy
  activation pulls the ACT table loads ahead of sqrt0.
DMA notes: per-queue throughput is ~85GB/s, issue costs ~0.8-1.6us of
sequencer time per dma_start (so 2 transfers/queue max), and only
SP/Activation issue usable HWDGE queues (gpsimd SWDGE moved 64KB in
~4us). Byte-balanced schedule, bank-0 operands first on each queue:
  sync:   A = xt(s0-4)  42KB   then  B = xt(s5-12)           67KB
  scalar: C = ct(s0-4)  21KB   then  D = ct(s5-15)|xt(s13-15) 71KB
"""

import numpy as np

B, K, S, F = 128, 64, 128, 64
NCORES = 8
SLOC = S // NCORES          # 16 sequence positions per core
BANKS = (5, 5, 4, 2)        # skewed psum banks; short final bank
CP = F + 2                  # 66 contraction rows (data + aug)
P0 = 5                      # s-positions in the first piece (bank 0)
XB = 13                     # xt(s5-12) in B; xt(s13-15) rides in D

X8_DT = "float8e4"
DI_DT = "float16"           # sqrt results + partial sums; 2x DVE throughput

_CACHE = {}

# D column layout (fp8): ct blocks for s5-15, then xt blocks for s13-15
D_CT_N = SLOC - P0          # 11 ct blocks of K
D_XT_OFF = D_CT_N * K       # 704
D_COLS = D_XT_OFF + (SLOC - XB) * B  # 704 + 384 = 1088


def _operand_layout(s):
    """Returns ((xt_tensor, xt_col), (ct_tensor, ct_col)) for position s."""
    if s < P0:
        xt = ("A", s * B)
        ct = ("C", s * K)
    else:
        ct = ("D", (s - P0) * K)
        if s < XB:
            xt = ("B", (s - P0) * B)
        else:
            xt = ("D", D_XT_OFF + (s - XB) * B)
    return xt, ct


def _build_nc():
    import concourse.bacc as bacc
    import concourse.tile as tile
    from concourse import mybir

    f32 = mybir.dt.float32
    f8 = getattr(mybir.dt, X8_DT)
    fdi = getattr(mybir.dt, DI_DT)
    nc = bacc.Bacc("TRN2", target_bir_lowering=False, debug=False)

    a_d = nc.dram_tensor("A", [CP, P0 * B], f8, kind="ExternalInput")
    b_d = nc.dram_tensor("B", [CP, (XB - P0) * B], f8, kind="ExternalInput")
    c_d = nc.dram_tensor("C", [CP, P0 * K], f8, kind="ExternalInput")
    d_d = nc.dram_tensor("D", [CP, D_COLS], f8, kind="ExternalInput")
    qp0_d = nc.dram_tensor("qp0", [B, K], fdi, kind="ExternalOutput")
    qp1_d = nc.dram_tensor("qp1", [B, K], fdi, kind="ExternalOutput")
    qp2_d = nc.dram_tensor("qp2", [B, K], fdi, kind="ExternalOutput")

    with tile.TileContext(nc) as tc:
        with (
            tc.tile_pool(name="ins", bufs=1) as in_pool,
            tc.tile_pool(name="psum", bufs=1, space="PSUM") as psum_pool,
            tc.tile_pool(name="work", bufs=1) as work_pool,
        ):
            # Dummy activation first: pulls the ACT table loads to the
            # top of the scalar stream, ahead of sqrt0's need.
            dm = work_pool.tile([1, 2], f32, name="dm")
            nc.vector.memset(dm[:], 1.0)
            dm2 = work_pool.tile([1, 2], f32, name="dm2")
            nc.scalar.activation(
                dm2[:], dm[:], mybir.ActivationFunctionType.Sqrt
            )

            tiles = {
                "A": in_pool.tile([CP, P0 * B], f8, name="At"),
                "B": in_pool.tile([CP, (XB - P0) * B], f8, name="Bt"),
                "C": in_pool.tile([CP, P0 * K], f8, name="Ct"),
                "D": in_pool.tile([CP, D_COLS], f8, name="Dt"),
            }
            nc.sync.dma_start(out=tiles["A"][:], in_=a_d.ap())
            nc.scalar.dma_start(out=tiles["C"][:], in_=c_d.ap())
            nc.sync.dma_start(out=tiles["B"][:], in_=b_d.ap())
            nc.scalar.dma_start(out=tiles["D"][:], in_=d_d.ap())

            pss = [
                psum_pool.tile([128, t * K], f32, name=f"ps{b}")
                for b, t in enumerate(BANKS)
            ]
            dis = [
                work_pool.tile([128, t, K], fdi, name=f"di{b}")
                for b, t in enumerate(BANKS)
            ]

            s = 0
            for b, t in enumerate(BANKS):
                for u in range(t):
                    (xn, xo), (cn, co) = _operand_layout(s)
                    nc.tensor.matmul(
                        pss[b][:, u * K:(u + 1) * K],
                        lhsT=tiles[xn][:, xo:xo + B],
                        rhs=tiles[cn][:, co:co + K],
                        start=True,
                        stop=True,
                    )
                    s += 1

            # per-bank: sqrt (ACT) + contiguous fp16 add-tree (DVE)
            def bank_tree(b, t):
                nc.scalar.activation(
                    dis[b][:], pss[b][:], mybir.ActivationFunctionType.Sqrt
                )
                d = dis[b]
                if t == 2:
                    pb = work_pool.tile([128, K], fdi, name=f"pb{b}")
                    nc.vector.tensor_tensor(
                        pb[:], d[:, 0, :], d[:, 1, :], op=mybir.AluOpType.add
                    )
                    return pb
                # t in (4, 5): pairwise halves then fold the odd tail
                tb = work_pool.tile([128, 2, K], fdi, name=f"tb{b}")
                nc.vector.tensor_tensor(
                    tb[:], d[:, 0:2, :], d[:, 2:4, :], op=mybir.AluOpType.add
                )
                pb = work_pool.tile([128, K], fdi, name=f"pb{b}")
                nc.vector.tensor_tensor(
                    pb[:], tb[:, 0, :], tb[:, 1, :], op=mybir.AluOpType.add
                )
                if t == 5:
                    pb5 = work_pool.tile([128, K], fdi, name=f"pb5{b}")
                    nc.vector.tensor_tensor(
                        pb5[:], pb[:], d[:, 4, :], op=mybir.AluOpType.add
                    )
                    pb = pb5
                return pb

            # Three output partials, each DMA'd as soon as its input
            # lands, so no accumulator add sits on the terminal chain:
            # qp0 = banks 0+1 (early), qp1 = bank 2, qp2 = bank 3.
            pb0 = bank_tree(0, BANKS[0])
            pb1 = bank_tree(1, BANKS[1])
            a01 = work_pool.tile([128, K], fdi, name="a01")
            nc.vector.tensor_tensor(
                a01[:], pb0[:], pb1[:], op=mybir.AluOpType.add
            )
            nc.scalar.dma_start(out=qp0_d.ap(), in_=a01[:])
            pb2 = bank_tree(2, BANKS[2])
            nc.sync.dma_start(out=qp1_d.ap(), in_=pb2[:])
            pb3 = bank_tree(3, BANKS[3])
            nc.scalar.dma_start(out=qp2_d.ap(), in_=pb3[:])

    nc.compile()
    return nc


def _prep_inputs(x, centroids):
    """Host-side shard + transpose + augmentation. Returns in_maps list."""
    from concourse import mybir

    f8_np = mybir.dt.np(getattr(mybir.dt, X8_DT))
    x = np.ascontiguousarray(np.asarray(x, dtype=np.float32)).reshape(B, S, F)
    c = np.ascontiguousarray(np.asarray(centroids, dtype=np.float32)).reshape(K, S, F)

    in_maps = []
    for i in range(NCORES):
        # full per-core xt [66, SLOC*B] and ct [66, SLOC*K] in f32 first
        sl = slice(i * SLOC, (i + 1) * SLOC)
        xs = x[:, sl, :]                              # [B, SLOC, F]
        xt = np.empty((CP, SLOC * B), dtype=np.float32)
        xt[:F] = xs.transpose(2, 1, 0).reshape(F, SLOC * B)
        xt[F] = 1.0
        xt[F + 1] = ((xs * xs).sum(-1, dtype=np.float32).T).reshape(SLOC * B)
        cs = c[:, sl, :]                              # [K, SLOC, F]
        ct = np.empty((CP, SLOC * K), dtype=np.float32)
        ct[:F] = (-2.0 * cs).transpose(2, 1, 0).reshape(F, SLOC * K)
        ct[F] = ((cs * cs).sum(-1, dtype=np.float32).T).reshape(SLOC * K)
        ct[F + 1] = 1.0
        xt8 = xt.astype(f8_np)
        ct8 = ct.astype(f8_np)

        dmat = np.empty((CP, D_COLS), dtype=f8_np)
        dmat[:, :D_XT_OFF] = ct8[:, P0 * K:]
        dmat[:, D_XT_OFF:] = xt8[:, XB * B:]
        in_maps.append({
            "A": np.ascontiguousarray(xt8[:, :P0 * B]),
            "B": np.ascontiguousarray(xt8[:, P0 * B:XB * B]),
            "C": np.ascontiguousarray(ct8[:, :P0 * K]),
            "D": dmat,
        })
    return in_maps


def kernel(x, centroids):
    from concourse.bass_utils import run_bass_kernel_spmd

    if "nc" not in _CACHE:
        _CACHE["nc"] = _build_nc()
    nc = _CACHE["nc"]

    in_maps = _prep_inputs(x, centroids)
    res = run_bass_kernel_spmd(nc, in_maps, core_ids=list(range(NCORES)))
    dist = np.zeros((B, K), dtype=np.float64)
    for i in range(NCORES):
        dist += res.results[i]["qp0"].astype(np.float64)
        dist += res.results[i]["qp1"].astype(np.float64)
        dist += res.results[i]["qp2"].astype(np.float64)
    # q tail (exact, host): q = (1 + d^2/2)^-3 normalized over k
    q = 1.0 / (1.0 + dist * dist / 2.0)
    q = q * q * q
    q = q / q.sum(axis=1, keepdims=True)
    return q.astype(np.float32)


# revision 16
# speedup vs baseline: 1.0300x; 1.0300x over previous
"""Trainium2 Bass kernel for nn_Clustering_80900003987951 (vq_codebook).

Math (reference):
  x: [B=128, S=128, F=64, 1], centroids: [1, K=64, S=128, F=64]
  d2[b,k,s] = sum_f (x[b,s,f] - c[k,s,f])^2
  dist[b,k] = sum_s sqrt(d2[b,k,s])
  q = (1 + dist^2/2)^-3, normalized over k                  -> [B, K]

Strategy: shard the SEQUENCE dim across the 8 cores (S_loc=16), keep the
full batch on every core. Per-core input drops to ~200KB (vs 1.36MB for
batch sharding, where every core must load all centroids), matmuls use
all 128 output partitions, and the device returns two per-core partial
sums of sqrt(d2) over its s-shard (split so the final DMA depends only
on the short last PSUM bank). The host sums the 16 partials and applies
the tiny q tail (25K flops, ~0.002% of the work) exactly in float64.

Device pipeline per core:
  xt [66, *]: rows 0-63 = x^T (F on partitions), 64 = 1, 65 = |x|^2
  ct [66, *]: rows 0-63 = -2*c^T, 64 = |c|^2, 65 = 1
  per s: d2 tile = xt_s^T @ ct_s -> PSUM [128,64], ONE fp8 matmul per s
  (uniform weight dtype — alternating fp8/fp16 weights measured 325ns/s
  because it breaks ldweights/matmul pipelining, vs 60ns uniform; the
  part of the fp8 quantization error that is constant across k cancels
  in the normalized output; measured 6e-3 vs the 2e-2 budget).
  16 s split over 4 PSUM banks, skewed (5,5,4,2) so after the final
  matmul only sqrt[128,128] + one DVE pair-add + its own DMA remain; ACT
  sqrt per bank -> fp16 sbuf; contiguous fp16 add-trees + accumulator on
  DVE (strided tensor_reduce measured 1.8ns/elem vs ~0.5 here). A dummy
  activation pulls the ACT table loads ahead of sqrt0.
DMA notes: per-queue throughput is ~85GB/s, issue costs ~0.8-1.6us of
sequencer time per dma_start (so 2 transfers/queue max), and only
SP/Activation issue usable HWDGE queues (gpsimd SWDGE moved 64KB in
~4us). Byte-balanced schedule, bank-0 operands first on each queue:
  sync:   A = xt(s0-4)  42KB   then  B = xt(s5-12)           67KB
  scalar: C = ct(s0-4)  21KB   then  D = ct(s5-15)|xt(s13-15) 71KB
"""

import numpy as np

B, K, S, F = 128, 64, 128, 64
NCORES = 8
SLOC = S // NCORES          # 16 sequence positions per core
BANKS = (5, 5, 4, 2)        # skewed psum banks; short final bank
CP = F + 2                  # 66 contraction rows (data + aug)
P0 = 5                      # s-positions in the first piece (bank 0)
XB = 13                     # xt(s5-12) in B; xt(s13-15) rides in D

X8_DT = "float8e4"
DI_DT = "float16"           # sqrt results + partial sums; 2x DVE throughput

_CACHE = {}

# D column layout (fp8): ct blocks for s5-15, then xt blocks for s13-15
D_CT_N = SLOC - P0          # 11 ct blocks of K
D_XT_OFF = D_CT_N * K       # 704
D_COLS = D_XT_OFF + (SLOC - XB) * B  # 704 + 384 = 1088


def _operand_layout(s):
    """Returns ((xt_tensor, xt_col), (ct_tensor, ct_col)) for position s."""
    if s < P0:
        xt = ("A", s * B)
        ct = ("C", s * K)
    else:
        ct = ("D", (s - P0) * K)
        if s < XB:
            xt = ("B", (s - P0) * B)
        else:
            xt = ("D", D_XT_OFF + (s - XB) * B)
    return xt, ct


def _build_nc():
    import concourse.bacc as bacc
    import concourse.tile as tile
    from concourse import mybir

    f32 = mybir.dt.float32
    f8 = getattr(mybir.dt, X8_DT)
    fdi = getattr(mybir.dt, DI_DT)
    nc = bacc.Bacc("TRN2", target_bir_lowering=False, debug=False)

    a_d = nc.dram_tensor("A", [CP, P0 * B], f8, kind="ExternalInput")
    b_d = nc.dram_tensor("B", [CP, (XB - P0) * B], f8, kind="ExternalInput")
    c_d = nc.dram_tensor("C", [CP, P0 * K], f8, kind="ExternalInput")
    d_d = nc.dram_tensor("D", [CP, D_COLS], f8, kind="ExternalInput")
    qp0_d = nc.dram_tensor("qp0", [B, K], fdi, kind="ExternalOutput")
    qp1_d = nc.dram_tensor("qp1", [B, K], fdi, kind="ExternalOutput")

    with tile.TileContext(nc) as tc:
        with (
            tc.tile_pool(name="ins", bufs=1) as in_pool,
            tc.tile_pool(name="psum", bufs=1, space="PSUM") as psum_pool,
            tc.tile_pool(name="work", bufs=1) as work_pool,
        ):
            # Dummy activation first: pulls the ACT table loads to the
            # top of the scalar stream, ahead of sqrt0's need.
            dm = work_pool.tile([1, 2], f32, name="dm")
            nc.vector.memset(dm[:], 1.0)
            dm2 = work_pool.tile([1, 2], f32, name="dm2")
            nc.scalar.activation(
                dm2[:], dm[:], mybir.ActivationFunctionType.Sqrt
            )

            tiles = {
                "A": in_pool.tile([CP, P0 * B], f8, name="At"),
                "B": in_pool.tile([CP, (XB - P0) * B], f8, name="Bt"),
                "C": in_pool.tile([CP, P0 * K], f8, name="Ct"),
                "D": in_pool.tile([CP, D_COLS], f8, name="Dt"),
            }
            nc.sync.dma_start(out=tiles["A"][:], in_=a_d.ap())
            nc.scalar.dma_start(out=tiles["C"][:], in_=c_d.ap())
            nc.sync.dma_start(out=tiles["B"][:], in_=b_d.ap())
            nc.scalar.dma_start(out=tiles["D"][:], in_=d_d.ap())

            pss = [
                psum_pool.tile([128, t * K], f32, name=f"ps{b}")
                for b, t in enumerate(BANKS)
            ]
            dis = [
                work_pool.tile([128, t, K], fdi, name=f"di{b}")
                for b, t in enumerate(BANKS)
            ]

            s = 0
            for b, t in enumerate(BANKS):
                for u in range(t):
                    (xn, xo), (cn, co) = _operand_layout(s)
                    nc.tensor.matmul(
                        pss[b][:, u * K:(u + 1) * K],
                        lhsT=tiles[xn][:, xo:xo + B],
                        rhs=tiles[cn][:, co:co + K],
                        start=True,
                        stop=True,
                    )
                    s += 1

            # per-bank: sqrt (ACT) + contiguous fp16 add-tree (DVE)
            def bank_tree(b, t):
                nc.scalar.activation(
                    dis[b][:], pss[b][:], mybir.ActivationFunctionType.Sqrt
                )
                d = dis[b]
                if t == 2:
                    pb = work_pool.tile([128, K], fdi, name=f"pb{b}")
                    nc.vector.tensor_tensor(
                        pb[:], d[:, 0, :], d[:, 1, :], op=mybir.AluOpType.add
                    )
                    return pb
                # t in (4, 5): pairwise halves then fold the odd tail
                tb = work_pool.tile([128, 2, K], fdi, name=f"tb{b}")
                nc.vector.tensor_tensor(
                    tb[:], d[:, 0:2, :], d[:, 2:4, :], op=mybir.AluOpType.add
                )
                pb = work_pool.tile([128, K], fdi, name=f"pb{b}")
                nc.vector.tensor_tensor(
                    pb[:], tb[:, 0, :], tb[:, 1, :], op=mybir.AluOpType.add
                )
                if t == 5:
                    pb5 = work_pool.tile([128, K], fdi, name=f"pb5{b}")
                    nc.vector.tensor_tensor(
                        pb5[:], pb[:], d[:, 4, :], op=mybir.AluOpType.add
                    )
                    pb = pb5
                return pb

            # banks 0-2 fold into qp0 (scalar queue, overlapped); the
            # terminal chain is only: sqrt(bank3) -> pair add -> qp1 DMA.
            # (a 3-output variant with one DMA per bank measured 0.5us
            # SLOWER — extra issue time + completion sems on the tail.)
            pb0 = bank_tree(0, BANKS[0])
            pb1 = bank_tree(1, BANKS[1])
            a01 = work_pool.tile([128, K], fdi, name="a01")
            nc.vector.tensor_tensor(
                a01[:], pb0[:], pb1[:], op=mybir.AluOpType.add
            )
            pb2 = bank_tree(2, BANKS[2])
            a012 = work_pool.tile([128, K], fdi, name="a012")
            nc.vector.tensor_tensor(
                a012[:], a01[:], pb2[:], op=mybir.AluOpType.add
            )
            nc.scalar.dma_start(out=qp0_d.ap(), in_=a012[:])
            pb3 = bank_tree(3, BANKS[3])
            nc.sync.dma_start(out=qp1_d.ap(), in_=pb3[:])

    nc.compile()
    return nc


def _prep_inputs(x, centroids):
    """Host-side shard + transpose + augmentation. Returns in_maps list."""
    from concourse import mybir

    f8_np = mybir.dt.np(getattr(mybir.dt, X8_DT))
    x = np.ascontiguousarray(np.asarray(x, dtype=np.float32)).reshape(B, S, F)
    c = np.ascontiguousarray(np.asarray(centroids, dtype=np.float32)).reshape(K, S, F)

    in_maps = []
    for i in range(NCORES):
        # full per-core xt [66, SLOC*B] and ct [66, SLOC*K] in f32 first
        sl = slice(i * SLOC, (i + 1) * SLOC)
        xs = x[:, sl, :]                              # [B, SLOC, F]
        xt = np.empty((CP, SLOC * B), dtype=np.float32)
        xt[:F] = xs.transpose(2, 1, 0).reshape(F, SLOC * B)
        xt[F] = 1.0
        xt[F + 1] = ((xs * xs).sum(-1, dtype=np.float32).T).reshape(SLOC * B)
        cs = c[:, sl, :]                              # [K, SLOC, F]
        ct = np.empty((CP, SLOC * K), dtype=np.float32)
        ct[:F] = (-2.0 * cs).transpose(2, 1, 0).reshape(F, SLOC * K)
        ct[F] = ((cs * cs).sum(-1, dtype=np.float32).T).reshape(SLOC * K)
        ct[F + 1] = 1.0
        xt8 = xt.astype(f8_np)
        ct8 = ct.astype(f8_np)

        dmat = np.empty((CP, D_COLS), dtype=f8_np)
        dmat[:, :D_XT_OFF] = ct8[:, P0 * K:]
        dmat[:, D_XT_OFF:] = xt8[:, XB * B:]
        in_maps.append({
            "A": np.ascontiguousarray(xt8[:, :P0 * B]),
            "B": np.ascontiguousarray(xt8[:, P0 * B:XB * B]),
            "C": np.ascontiguousarray(ct8[:, :P0 * K]),
            "D": dmat,
        })
    return in_maps


def kernel(x, centroids):
    from concourse.bass_utils import run_bass_kernel_spmd

    if "nc" not in _CACHE:
        _CACHE["nc"] = _build_nc()
    nc = _CACHE["nc"]

    in_maps = _prep_inputs(x, centroids)
    res = run_bass_kernel_spmd(nc, in_maps, core_ids=list(range(NCORES)))
    dist = np.zeros((B, K), dtype=np.float64)
    for i in range(NCORES):
        dist += res.results[i]["qp0"].astype(np.float64)
        dist += res.results[i]["qp1"].astype(np.float64)
    # q tail (exact, host): q = (1 + d^2/2)^-3 normalized over k
    q = 1.0 / (1.0 + dist * dist / 2.0)
    q = q * q * q
    q = q / q.sum(axis=1, keepdims=True)
    return q.astype(np.float32)
